# revision 4
# baseline (speedup 1.0000x reference)
"""MetaRoPE kernel for Trainium2, 8 NeuronCores — fp16 I/O, 2x-mode DVE,
merged muls, partial GpSimd offload.

Reference computation:
    r = rotate_m[token_positions]            # [S, D, D], block-diag 2x2 rotations
    out = einsum('bhsi,soi->bhso', x, r)     # x: [4, 32, 4096, 64] fp32

Because r is block-diagonal with 2x2 blocks, out = x * A + pairswap(x * B')
with host-precomputed tables A, B' of shape [S, D] (see _tables).

Precision/bandwidth: the harness gate is rel_err < 2e-2; fp16 end-to-end
(host converts x fp32->fp16, device computes in fp16, host converts the
fp16 result back) measures ~1.1e-3 and halves both HBM traffic and DVE
element cost. Plain InstTensorTensor ops hit the DVE 2x_1p perf mode with
packed fp16 (~0.49 ns/elem/partition measured, even with the stride -1
pair-swap operand); scalar_tensor_tensor would disable all perf modes.

Sharding: x reshaped to [128 (b,h) slabs, 4096, 64]; 16 slabs per core.
Each slab [4096*64] is viewed as [128 partitions, 2048 free] (contiguous per
partition; partition p holds positions 32p..32p+31). Tables are replicated
per core as one [128, 2*FREE] fp16 tile (tb | ta) matching that layout.

Per core the 16 slabs are processed in chunks (CHUNK_PLAN, tapered small at
the ends). Each chunk: one load (sync ring); ONE merged DVE multiply
computing u = x*tb and o = x*ta into one [128, 2*cfree] tile (x broadcast
via a step-0 AP dim, tables broadcast across slabs); one pair-swapped
in-place add o += pairswap(u) split by columns between DVE and GpSimd
(POOL_ADD_FRAC); one store (scalar ring). Steady state aims DMA-bound
(~17.9 MB/core at ~360 GB/s => ~50 us) with DVE (~38 us) + GpSimd (~16 us)
hidden underneath.
"""

import sys

import numpy as np

_TRN_REPO = "/opt/trn_rl_repo"
if _TRN_REPO not in sys.path:
    sys.path.insert(0, _TRN_REPO)

B, H, S, D = 4, 32, 4096, 64
BH = B * H                      # 128 (b,h) slabs
N_CORES = 8
BH_PER_CORE = BH // N_CORES     # 16 slabs per core
FREE = (S // 128) * D           # 2048 free elements per partition per slab
ROWS = BH_PER_CORE * 128        # 2048 dram rows per core, [ROWS, FREE] fp16
# slabs per chunk, tapered: small first chunk so compute starts early,
# small last chunk so the final store is short
CHUNK_PLAN = [1, 1, 2, 2, 2, 2, 2, 2, 1, 1]
assert sum(CHUNK_PLAN) == BH_PER_CORE
XIN_BUFS = 5
W_BUFS = 3
# fraction of each middle-chunk ADD's columns handed to GpSimd (Pool);
# DVE handles the rest. 0 disables the offload.
POOL_ADD_FRAC = 0.5

_prog_cache = {}


def _build_program():
    """Build (and cache) the SPMD Bass program for one core."""
    if "nc" in _prog_cache:
        return _prog_cache["nc"]

    import concourse.bacc as bacc
    import concourse.bass as bass
    import concourse.mybir as mybir
    import concourse.tile as tile

    f16 = mybir.dt.float16
    nc = bacc.Bacc(
        "TRN2", target_bir_lowering=False, debug=False, num_devices=N_CORES
    )
    x_d = nc.dram_tensor("x", [ROWS, FREE], f16, kind="ExternalInput").ap()
    ta_d = nc.dram_tensor("ta", [128, FREE], f16, kind="ExternalInput").ap()
    tb_d = nc.dram_tensor("tb", [128, FREE], f16, kind="ExternalInput").ap()
    o_d = nc.dram_tensor("out", [ROWS, FREE], f16, kind="ExternalOutput").ap()

    with tile.TileContext(nc) as tc:
        with (
            tc.tile_pool(name="tabs", bufs=1) as tabs,
            tc.tile_pool(name="xin", bufs=XIN_BUFS) as xin,
            tc.tile_pool(name="w", bufs=W_BUFS) as wpool,
        ):
            # one combined table tile: tb in [0:FREE), ta in [FREE:2*FREE)
            tt = tabs.tile([128, 2 * FREE], f16)
            hf = FREE // 2
            # table loads on the scalar ring (idle at start), halves ordered
            # so the first half-slab compute (needs tb+ta cols [0:hf)) can
            # start asap while the sync ring pulls the first x chunk
            nc.scalar.dma_start(tt[:, 0:hf], tb_d[:, 0:hf])
            nc.scalar.dma_start(tt[:, FREE : FREE + hf], ta_d[:, 0:hf])
            nc.scalar.dma_start(tt[:, hf:FREE], tb_d[:, hf:])
            nc.scalar.dma_start(tt[:, FREE + hf :], ta_d[:, hf:])

            def merged_mul(w, xt, nsl, fs=None):
                """One DVE multiply: w[:, two, j, f] = x[j, f] * (tb|ta)[f].

                fs: optional (start, size) column window within each slab
                (head/tail half-chunks); nsl must be 1 in that case."""
                if fs is None:
                    cfree = nsl * FREE
                    w_ap = w[:].rearrange(
                        "p (two j f) -> p two j f", two=2, j=nsl
                    )
                    x3 = xt[:].rearrange("p (j f) -> p j f", j=nsl)
                    x_b = bass.AP(
                        x3.tensor, x3.offset,
                        [x3.ap[0], [0, 2], x3.ap[1], x3.ap[2]],
                    )
                    t2 = tt[:].rearrange("p (two f) -> p two f", two=2)
                    t_b = bass.AP(
                        t2.tensor, t2.offset,
                        [t2.ap[0], t2.ap[1], [0, nsl], t2.ap[2]],
                    )
                else:
                    assert nsl == 1
                    lo, sz = fs
                    cfree = FREE
                    w_ap = w[:].rearrange("p (two f) -> p two f", two=2)[
                        :, :, lo : lo + sz
                    ]
                    xs = xt[:, lo : lo + sz]
                    x_b = bass.AP(
                        xs.tensor, xs.offset, [xs.ap[0], [0, 2], xs.ap[1]]
                    )
                    t_b = tt[:].rearrange("p (two f) -> p two f", two=2)[
                        :, :, lo : lo + sz
                    ]
                nc.vector.tensor_mul(w_ap, x_b, t_b)
                return cfree

            def swap_add(w, cfree, lo, sz, pool_cols):
                """o[:, lo:lo+sz] += pairswap(u[:, lo:lo+sz]); the last
                pool_cols columns go to GpSimd, the rest to DVE."""
                u_ap = w[:, lo : lo + sz]
                o_ap = w[:, cfree + lo : cfree + lo + sz]
                usw = u_ap.rearrange("p (n two) -> p n two", two=2)[:, :, ::-1]
                os3 = o_ap.rearrange("p (n two) -> p n two", two=2)
                n = sz // 2
                np_pool = pool_cols // 2
                nd = n - np_pool
                if nd:
                    nc.vector.tensor_add(
                        os3[:, :nd, :], os3[:, :nd, :], usw[:, :nd, :]
                    )
                if np_pool:
                    nc.gpsimd.tensor_add(
                        os3[:, nd:, :], os3[:, nd:, :], usw[:, nd:, :]
                    )

            row0 = 0
            for ci, nsl in enumerate(CHUNK_PLAN):
                first = ci == 0
                last = ci == len(CHUNK_PLAN) - 1
                cfree = nsl * FREE
                rows = x_d[row0 * 128 : (row0 + nsl) * 128, :]
                src = rows.rearrange("(j p) f -> p j f", j=nsl)
                xt = xin.tile([128, cfree], f16, tag="xt")
                if first:
                    # split the first load so compute can start after 0.25 MiB
                    assert nsl == 1
                    h = cfree // 2
                    nc.sync.dma_start(xt[:, :h], rows[:, :h])
                    nc.sync.dma_start(xt[:, h:], rows[:, h:])
                else:
                    nc.sync.dma_start(
                        xt[:].rearrange("p (j f) -> p j f", j=nsl), src
                    )

                w = wpool.tile([128, 2 * cfree], f16, tag="w")
                orows = o_d[row0 * 128 : (row0 + nsl) * 128, :]

                if first or last:
                    # head/tail chunk: process in free-dim halves (head: start
                    # computing after the first half-load; tail: overlap the
                    # final store with the second half's compute)
                    assert nsl == 1
                    part = cfree // 2
                    for hi in range(2):
                        lo = hi * part
                        merged_mul(w, xt, 1, fs=(lo, part))
                        swap_add(w, cfree, lo, part, 0)
                        nc.scalar.dma_start(
                            orows[:, lo : lo + part],
                            w[:, cfree + lo : cfree + lo + part],
                        )
                else:
                    merged_mul(w, xt, nsl)
                    pool_cols = int(cfree * POOL_ADD_FRAC) & ~1
                    swap_add(w, cfree, 0, cfree, pool_cols)
                    dst = orows.rearrange("(j p) f -> p j f", j=nsl)
                    nc.scalar.dma_start(
                        dst,
                        w[:, cfree:].rearrange("p (j f) -> p j f", j=nsl),
                    )
                row0 += nsl

    nc.compile()
    _prog_cache["nc"] = nc
    return nc


def _default_rotate_m(theta=10000.0):
    """Rebuild the reference's rotation buffer if the harness doesn't pass it."""
    half = D // 2
    try:  # replicate the reference's jax-f32 arithmetic exactly if possible
        import jax.numpy as jnp

        pos = np.asarray(jnp.arange(S, dtype=jnp.float32))
        inv_freq = np.asarray(
            theta ** (-(2.0 * jnp.arange(half, dtype=jnp.float32)) / D)
        )
        ang = np.asarray(pos[:, None] * inv_freq[None, :], dtype=np.float32)
        c, s = np.asarray(jnp.cos(ang)), np.asarray(jnp.sin(ang))
    except Exception:
        pos = np.arange(S, dtype=np.float32)
        exp = (-(2.0 * np.arange(half, dtype=np.float32)) / D).astype(np.float32)
        inv_freq = np.power(np.float32(theta), exp, dtype=np.float32)
        ang = (pos[:, None] * inv_freq[None, :]).astype(np.float32)
        c, s = np.cos(ang, dtype=np.float32), np.sin(ang, dtype=np.float32)
    idx = 2 * np.arange(half)
    r = np.zeros((S, D, D), dtype=np.float32)
    r[:, idx, idx] = c
    r[:, idx, idx + 1] = -s
    r[:, idx + 1, idx] = s
    r[:, idx + 1, idx + 1] = c
    return r


def _tables(token_positions, rotate_m):
    """Host-precompute the [128, FREE] fp16 A and B' tables.

    A[s,2k] = r[2k,2k], A[s,2k+1] = r[2k+1,2k+1]  (cos terms)
    B'[s,2k] = r[2k+1,2k], B'[s,2k+1] = r[2k,2k+1] (pre-pairswapped sin terms
    so that pairswap(x*B') lands the right products on the right lanes)."""
    if rotate_m is None:
        rotate_m = _default_rotate_m()
    r = np.asarray(rotate_m, dtype=np.float32)[np.asarray(token_positions)]
    idx = np.arange(D // 2) * 2
    a = r[:, idx, idx]            # x_even -> out_even
    b = r[:, idx, idx + 1]        # x_odd  -> out_even
    c = r[:, idx + 1, idx + 1]    # x_odd  -> out_odd
    d = r[:, idx + 1, idx]        # x_even -> out_odd
    A = np.empty((S, D), np.float32)
    A[:, 0::2] = a
    A[:, 1::2] = c
    Bp = np.empty((S, D), np.float32)
    Bp[:, 0::2] = d
    Bp[:, 1::2] = b
    return (
        np.ascontiguousarray(A.reshape(128, FREE)).astype(np.float16),
        np.ascontiguousarray(Bp.reshape(128, FREE)).astype(np.float16),
    )


def _in_maps(x, token_positions, rotate_m):
    ta, tb = _tables(token_positions, rotate_m)
    xs = np.asarray(x, dtype=np.float32).astype(np.float16).reshape(
        N_CORES, ROWS, FREE
    )
    xs = np.ascontiguousarray(xs)
    return [{"x": xs[i], "ta": ta, "tb": tb} for i in range(N_CORES)]


def _run(x, token_positions, rotate_m=None, trace=False, trace_cores=None):
    from concourse.bass_utils import run_bass_kernel_spmd

    nc = _build_program()
    in_maps = _in_maps(x, token_positions, rotate_m)
    res = run_bass_kernel_spmd(
        nc,
        in_maps,
        list(range(N_CORES)),
        trace=trace,
        trace_cores=trace_cores,
    )
    out = np.concatenate(
        [res.results[i]["out"].reshape(1, ROWS * FREE) for i in range(N_CORES)]
    ).reshape(B, H, S, D).astype(np.float32)
    return out, res


def kernel(x, token_positions, rotate_m=None, **_unused):
    out, _ = _run(x, token_positions, rotate_m, trace=False)
    return out


# revision 5
# speedup vs baseline: 1.2135x; 1.2135x over previous
"""MetaRoPE kernel for Trainium2, 8 NeuronCores — fp16 I/O, 2x-mode DVE,
merged muls, partial GpSimd offload.

Reference computation:
    r = rotate_m[token_positions]            # [S, D, D], block-diag 2x2 rotations
    out = einsum('bhsi,soi->bhso', x, r)     # x: [4, 32, 4096, 64] fp32

Because r is block-diagonal with 2x2 blocks, out = x * A + pairswap(x * B')
with host-precomputed tables A, B' of shape [S, D] (see _tables).

Precision/bandwidth: the harness gate is rel_err < 2e-2; fp16 end-to-end
(host converts x fp32->fp16, device computes in fp16, host converts the
fp16 result back) measures ~1.1e-3 and halves both HBM traffic and DVE
element cost. Plain InstTensorTensor ops hit the DVE 2x_1p perf mode with
packed fp16 (~0.49 ns/elem/partition measured, even with the stride -1
pair-swap operand); scalar_tensor_tensor would disable all perf modes.

Sharding: x reshaped to [128 (b,h) slabs, 4096, 64]; 16 slabs per core.
Each slab [4096*64] is viewed as [128 partitions, 2048 free] (contiguous per
partition; partition p holds positions 32p..32p+31). Tables are replicated
per core as one [128, 2*FREE] fp16 tile (tb | ta) matching that layout.

Per core the 16 slabs are processed in chunks (CHUNK_PLAN, tapered small at
the ends). Each chunk: one load (sync ring); ONE merged DVE multiply
computing u = x*tb and o = x*ta into one [128, 2*cfree] tile (x broadcast
via a step-0 AP dim, tables broadcast across slabs); one pair-swapped
in-place add o += pairswap(u) split by columns between DVE and GpSimd
(POOL_ADD_FRAC); one store (scalar ring). Steady state aims DMA-bound
(~17.9 MB/core at ~360 GB/s => ~50 us) with DVE (~38 us) + GpSimd (~16 us)
hidden underneath.
"""

import sys

import numpy as np

_TRN_REPO = "/opt/trn_rl_repo"
if _TRN_REPO not in sys.path:
    sys.path.insert(0, _TRN_REPO)

B, H, S, D = 4, 32, 4096, 64
BH = B * H                      # 128 (b,h) slabs
N_CORES = 8
BH_PER_CORE = BH // N_CORES     # 16 slabs per core
FREE = (S // 128) * D           # 2048 free elements per partition per slab
ROWS = BH_PER_CORE * 128        # 2048 dram rows per core, [ROWS, FREE] fp16
# slabs per chunk, tapered: small first chunk so compute starts early,
# small last chunk so the final store is short; big middle chunks amortize
# DVE per-instruction startup (measured 0.38 ns/elem at 8192-elem ops vs
# 0.49 at 4096)
CHUNK_PLAN = [1, 2, 4, 4, 4, 1]
assert sum(CHUNK_PLAN) == BH_PER_CORE
XIN_BUFS = 3
W_BUFS = 3
# fraction of each middle-chunk ADD's columns handed to GpSimd (Pool);
# DVE handles the rest. 0 disables the offload (GpSimd measured 2.5 ns/elem
# with ~700ns semaphore handling — it becomes the straggler if given work).
POOL_ADD_FRAC = 0.0

_prog_cache = {}


def _build_program():
    """Build (and cache) the SPMD Bass program for one core."""
    if "nc" in _prog_cache:
        return _prog_cache["nc"]

    import concourse.bacc as bacc
    import concourse.bass as bass
    import concourse.mybir as mybir
    import concourse.tile as tile

    f16 = mybir.dt.float16
    nc = bacc.Bacc(
        "TRN2", target_bir_lowering=False, debug=False, num_devices=N_CORES
    )
    x_d = nc.dram_tensor("x", [ROWS, FREE], f16, kind="ExternalInput").ap()
    ta_d = nc.dram_tensor("ta", [128, FREE], f16, kind="ExternalInput").ap()
    tb_d = nc.dram_tensor("tb", [128, FREE], f16, kind="ExternalInput").ap()
    o_d = nc.dram_tensor("out", [ROWS, FREE], f16, kind="ExternalOutput").ap()

    with tile.TileContext(nc) as tc:
        with (
            tc.tile_pool(name="tabs", bufs=1) as tabs,
            tc.tile_pool(name="xin", bufs=XIN_BUFS) as xin,
            tc.tile_pool(name="w", bufs=W_BUFS) as wpool,
        ):
            # one combined table tile: tb in [0:FREE), ta in [FREE:2*FREE)
            tt = tabs.tile([128, 2 * FREE], f16)
            hf = FREE // 2
            # table loads on the scalar ring (idle at start), halves ordered
            # so the first half-slab compute (needs tb+ta cols [0:hf)) can
            # start asap while the sync ring pulls the first x chunk
            nc.scalar.dma_start(tt[:, 0:hf], tb_d[:, 0:hf])
            nc.scalar.dma_start(tt[:, FREE : FREE + hf], ta_d[:, 0:hf])
            nc.scalar.dma_start(tt[:, hf:FREE], tb_d[:, hf:])
            nc.scalar.dma_start(tt[:, FREE + hf :], ta_d[:, hf:])

            def merged_mul(w, xt, nsl, fs=None):
                """One DVE multiply: w[:, two, j, f] = x[j, f] * (tb|ta)[f].

                fs: optional (start, size) column window within each slab
                (head/tail half-chunks); nsl must be 1 in that case."""
                if fs is None:
                    cfree = nsl * FREE
                    w_ap = w[:].rearrange(
                        "p (two j f) -> p two j f", two=2, j=nsl
                    )
                    x3 = xt[:].rearrange("p (j f) -> p j f", j=nsl)
                    x_b = bass.AP(
                        x3.tensor, x3.offset,
                        [x3.ap[0], [0, 2], x3.ap[1], x3.ap[2]],
                    )
                    t2 = tt[:].rearrange("p (two f) -> p two f", two=2)
                    t_b = bass.AP(
                        t2.tensor, t2.offset,
                        [t2.ap[0], t2.ap[1], [0, nsl], t2.ap[2]],
                    )
                else:
                    assert nsl == 1
                    lo, sz = fs
                    cfree = FREE
                    w_ap = w[:].rearrange("p (two f) -> p two f", two=2)[
                        :, :, lo : lo + sz
                    ]
                    xs = xt[:, lo : lo + sz]
                    x_b = bass.AP(
                        xs.tensor, xs.offset, [xs.ap[0], [0, 2], xs.ap[1]]
                    )
                    t_b = tt[:].rearrange("p (two f) -> p two f", two=2)[
                        :, :, lo : lo + sz
                    ]
                nc.vector.tensor_mul(w_ap, x_b, t_b)
                return cfree

            def swap_add(w, cfree, lo, sz, pool_cols):
                """o[:, lo:lo+sz] += pairswap(u[:, lo:lo+sz]); the last
                pool_cols columns go to GpSimd, the rest to DVE."""
                u_ap = w[:, lo : lo + sz]
                o_ap = w[:, cfree + lo : cfree + lo + sz]
                usw = u_ap.rearrange("p (n two) -> p n two", two=2)[:, :, ::-1]
                os3 = o_ap.rearrange("p (n two) -> p n two", two=2)
                n = sz // 2
                np_pool = pool_cols // 2
                nd = n - np_pool
                if nd:
                    nc.vector.tensor_add(
                        os3[:, :nd, :], os3[:, :nd, :], usw[:, :nd, :]
                    )
                if np_pool:
                    nc.gpsimd.tensor_add(
                        os3[:, nd:, :], os3[:, nd:, :], usw[:, nd:, :]
                    )

            row0 = 0
            for ci, nsl in enumerate(CHUNK_PLAN):
                first = ci == 0
                last = ci == len(CHUNK_PLAN) - 1
                cfree = nsl * FREE
                rows = x_d[row0 * 128 : (row0 + nsl) * 128, :]
                src = rows.rearrange("(j p) f -> p j f", j=nsl)
                xt = xin.tile([128, cfree], f16, tag="xt")
                if first:
                    # split the first load so compute can start after 0.25 MiB
                    assert nsl == 1
                    h = cfree // 2
                    nc.sync.dma_start(xt[:, :h], rows[:, :h])
                    nc.sync.dma_start(xt[:, h:], rows[:, h:])
                else:
                    nc.sync.dma_start(
                        xt[:].rearrange("p (j f) -> p j f", j=nsl), src
                    )

                w = wpool.tile([128, 2 * cfree], f16, tag="w")
                orows = o_d[row0 * 128 : (row0 + nsl) * 128, :]

                if first or last:
                    # head/tail chunk: process in free-dim halves (head: start
                    # computing after the first half-load; tail: overlap the
                    # final store with the second half's compute)
                    assert nsl == 1
                    part = cfree // 2
                    for hi in range(2):
                        lo = hi * part
                        merged_mul(w, xt, 1, fs=(lo, part))
                        swap_add(w, cfree, lo, part, 0)
                        nc.scalar.dma_start(
                            orows[:, lo : lo + part],
                            w[:, cfree + lo : cfree + lo + part],
                        )
                else:
                    merged_mul(w, xt, nsl)
                    pool_cols = int(cfree * POOL_ADD_FRAC) & ~1
                    swap_add(w, cfree, 0, cfree, pool_cols)
                    dst = orows.rearrange("(j p) f -> p j f", j=nsl)
                    nc.scalar.dma_start(
                        dst,
                        w[:, cfree:].rearrange("p (j f) -> p j f", j=nsl),
                    )
                row0 += nsl

    nc.compile()
    _prog_cache["nc"] = nc
    return nc


def _default_rotate_m(theta=10000.0):
    """Rebuild the reference's rotation buffer if the harness doesn't pass it."""
    half = D // 2
    try:  # replicate the reference's jax-f32 arithmetic exactly if possible
        import jax.numpy as jnp

        pos = np.asarray(jnp.arange(S, dtype=jnp.float32))
        inv_freq = np.asarray(
            theta ** (-(2.0 * jnp.arange(half, dtype=jnp.float32)) / D)
        )
        ang = np.asarray(pos[:, None] * inv_freq[None, :], dtype=np.float32)
        c, s = np.asarray(jnp.cos(ang)), np.asarray(jnp.sin(ang))
    except Exception:
        pos = np.arange(S, dtype=np.float32)
        exp = (-(2.0 * np.arange(half, dtype=np.float32)) / D).astype(np.float32)
        inv_freq = np.power(np.float32(theta), exp, dtype=np.float32)
        ang = (pos[:, None] * inv_freq[None, :]).astype(np.float32)
        c, s = np.cos(ang, dtype=np.float32), np.sin(ang, dtype=np.float32)
    idx = 2 * np.arange(half)
    r = np.zeros((S, D, D), dtype=np.float32)
    r[:, idx, idx] = c
    r[:, idx, idx + 1] = -s
    r[:, idx + 1, idx] = s
    r[:, idx + 1, idx + 1] = c
    return r


def _tables(token_positions, rotate_m):
    """Host-precompute the [128, FREE] fp16 A and B' tables.

    A[s,2k] = r[2k,2k], A[s,2k+1] = r[2k+1,2k+1]  (cos terms)
    B'[s,2k] = r[2k+1,2k], B'[s,2k+1] = r[2k,2k+1] (pre-pairswapped sin terms
    so that pairswap(x*B') lands the right products on the right lanes)."""
    if rotate_m is None:
        rotate_m = _default_rotate_m()
    r = np.asarray(rotate_m, dtype=np.float32)[np.asarray(token_positions)]
    idx = np.arange(D // 2) * 2
    a = r[:, idx, idx]            # x_even -> out_even
    b = r[:, idx, idx + 1]        # x_odd  -> out_even
    c = r[:, idx + 1, idx + 1]    # x_odd  -> out_odd
    d = r[:, idx + 1, idx]        # x_even -> out_odd
    A = np.empty((S, D), np.float32)
    A[:, 0::2] = a
    A[:, 1::2] = c
    Bp = np.empty((S, D), np.float32)
    Bp[:, 0::2] = d
    Bp[:, 1::2] = b
    return (
        np.ascontiguousarray(A.reshape(128, FREE)).astype(np.float16),
        np.ascontiguousarray(Bp.reshape(128, FREE)).astype(np.float16),
    )


def _in_maps(x, token_positions, rotate_m):
    ta, tb = _tables(token_positions, rotate_m)
    xs = np.asarray(x, dtype=np.float32).astype(np.float16).reshape(
        N_CORES, ROWS, FREE
    )
    xs = np.ascontiguousarray(xs)
    return [{"x": xs[i], "ta": ta, "tb": tb} for i in range(N_CORES)]


def _run(x, token_positions, rotate_m=None, trace=False, trace_cores=None):
    from concourse.bass_utils import run_bass_kernel_spmd

    nc = _build_program()
    in_maps = _in_maps(x, token_positions, rotate_m)
    res = run_bass_kernel_spmd(
        nc,
        in_maps,
        list(range(N_CORES)),
        trace=trace,
        trace_cores=trace_cores,
    )
    out = np.concatenate(
        [res.results[i]["out"].reshape(1, ROWS * FREE) for i in range(N_CORES)]
    ).reshape(B, H, S, D).astype(np.float32)
    return out, res


def kernel(x, token_positions, rotate_m=None, **_unused):
    out, _ = _run(x, token_positions, rotate_m, trace=False)
    return out


# revision 6
# speedup vs baseline: 1.2535x; 1.0329x over previous
"""MetaRoPE kernel for Trainium2, 8 NeuronCores — fp16 I/O, 2x-mode DVE,
merged muls, partial GpSimd offload.

Reference computation:
    r = rotate_m[token_positions]            # [S, D, D], block-diag 2x2 rotations
    out = einsum('bhsi,soi->bhso', x, r)     # x: [4, 32, 4096, 64] fp32

Because r is block-diagonal with 2x2 blocks, out = x * A + pairswap(x * B')
with host-precomputed tables A, B' of shape [S, D] (see _tables).

Precision/bandwidth: the harness gate is rel_err < 2e-2; fp16 end-to-end
(host converts x fp32->fp16, device computes in fp16, host converts the
fp16 result back) measures ~1.1e-3 and halves both HBM traffic and DVE
element cost. Plain InstTensorTensor ops hit the DVE 2x_1p perf mode with
packed fp16 (~0.49 ns/elem/partition measured, even with the stride -1
pair-swap operand); scalar_tensor_tensor would disable all perf modes.

Sharding: x reshaped to [128 (b,h) slabs, 4096, 64]; 16 slabs per core.
Each slab [4096*64] is viewed as [128 partitions, 2048 free] (contiguous per
partition; partition p holds positions 32p..32p+31). Tables are replicated
per core as one [128, 2*FREE] fp16 tile (tb | ta) matching that layout.

Per core the 16 slabs are processed in chunks (CHUNK_PLAN, tapered small at
the ends). Each chunk: one load (sync ring); ONE merged DVE multiply
computing u = x*tb and o = x*ta into one [128, 2*cfree] tile (x broadcast
via a step-0 AP dim, tables broadcast across slabs); one pair-swapped
in-place add o += pairswap(u) split by columns between DVE and GpSimd
(POOL_ADD_FRAC); one store (scalar ring). Steady state aims DMA-bound
(~17.9 MB/core at ~360 GB/s => ~50 us) with DVE (~38 us) + GpSimd (~16 us)
hidden underneath.
"""

import sys

import numpy as np

_TRN_REPO = "/opt/trn_rl_repo"
if _TRN_REPO not in sys.path:
    sys.path.insert(0, _TRN_REPO)

B, H, S, D = 4, 32, 4096, 64
BH = B * H                      # 128 (b,h) slabs
N_CORES = 8
BH_PER_CORE = BH // N_CORES     # 16 slabs per core
FREE = (S // 128) * D           # 2048 free elements per partition per slab
ROWS = BH_PER_CORE * 128        # 2048 dram rows per core, [ROWS, FREE] fp16
# slabs per chunk, tapered: small first chunk so compute starts early,
# small last chunk so the final store is short; big middle chunks amortize
# DVE per-instruction startup (measured 0.38 ns/elem at 8192-elem ops vs
# 0.49 at 4096)
CHUNK_PLAN = [1, 1, 2, 2, 2, 2, 2, 2, 1, 1]
assert sum(CHUNK_PLAN) == BH_PER_CORE
XIN_BUFS = 5
W_BUFS = 4
# fraction of each middle-chunk ADD's columns handed to GpSimd (Pool);
# DVE handles the rest. 0 disables the offload (GpSimd measured 2.5 ns/elem
# with ~700ns semaphore handling — it becomes the straggler if given work).
POOL_ADD_FRAC = 0.0

_prog_cache = {}


def _build_program():
    """Build (and cache) the SPMD Bass program for one core."""
    if "nc" in _prog_cache:
        return _prog_cache["nc"]

    import concourse.bacc as bacc
    import concourse.bass as bass
    import concourse.mybir as mybir
    import concourse.tile as tile

    f16 = mybir.dt.float16
    nc = bacc.Bacc(
        "TRN2", target_bir_lowering=False, debug=False, num_devices=N_CORES
    )
    x_d = nc.dram_tensor("x", [ROWS, FREE], f16, kind="ExternalInput").ap()
    ta_d = nc.dram_tensor("ta", [128, FREE], f16, kind="ExternalInput").ap()
    tb_d = nc.dram_tensor("tb", [128, FREE], f16, kind="ExternalInput").ap()
    o_d = nc.dram_tensor("out", [ROWS, FREE], f16, kind="ExternalOutput").ap()

    with tile.TileContext(nc) as tc:
        with (
            tc.tile_pool(name="tabs", bufs=1) as tabs,
            tc.tile_pool(name="xin", bufs=XIN_BUFS) as xin,
            tc.tile_pool(name="w", bufs=W_BUFS) as wpool,
        ):
            # one combined table tile: tb in [0:FREE), ta in [FREE:2*FREE)
            tt = tabs.tile([128, 2 * FREE], f16)
            hf = FREE // 2
            # table loads on the scalar ring (idle at start), halves ordered
            # so the first half-slab compute (needs tb+ta cols [0:hf)) can
            # start asap while the sync ring pulls the first x chunk
            nc.scalar.dma_start(tt[:, 0:hf], tb_d[:, 0:hf])
            nc.scalar.dma_start(tt[:, FREE : FREE + hf], ta_d[:, 0:hf])
            nc.scalar.dma_start(tt[:, hf:FREE], tb_d[:, hf:])
            nc.scalar.dma_start(tt[:, FREE + hf :], ta_d[:, hf:])

            def merged_mul(w, xt, nsl, fs=None):
                """One DVE multiply: w[:, two, j, f] = x[j, f] * (tb|ta)[f].

                fs: optional (start, size) column window within each slab
                (head/tail half-chunks); nsl must be 1 in that case."""
                if fs is None:
                    cfree = nsl * FREE
                    w_ap = w[:].rearrange(
                        "p (two j f) -> p two j f", two=2, j=nsl
                    )
                    x3 = xt[:].rearrange("p (j f) -> p j f", j=nsl)
                    x_b = bass.AP(
                        x3.tensor, x3.offset,
                        [x3.ap[0], [0, 2], x3.ap[1], x3.ap[2]],
                    )
                    t2 = tt[:].rearrange("p (two f) -> p two f", two=2)
                    t_b = bass.AP(
                        t2.tensor, t2.offset,
                        [t2.ap[0], t2.ap[1], [0, nsl], t2.ap[2]],
                    )
                else:
                    assert nsl == 1
                    lo, sz = fs
                    cfree = FREE
                    w_ap = w[:].rearrange("p (two f) -> p two f", two=2)[
                        :, :, lo : lo + sz
                    ]
                    xs = xt[:, lo : lo + sz]
                    x_b = bass.AP(
                        xs.tensor, xs.offset, [xs.ap[0], [0, 2], xs.ap[1]]
                    )
                    t_b = tt[:].rearrange("p (two f) -> p two f", two=2)[
                        :, :, lo : lo + sz
                    ]
                nc.vector.tensor_mul(w_ap, x_b, t_b)
                return cfree

            def swap_add(w, cfree, lo, sz, pool_cols):
                """o[:, lo:lo+sz] += pairswap(u[:, lo:lo+sz]); the last
                pool_cols columns go to GpSimd, the rest to DVE."""
                u_ap = w[:, lo : lo + sz]
                o_ap = w[:, cfree + lo : cfree + lo + sz]
                usw = u_ap.rearrange("p (n two) -> p n two", two=2)[:, :, ::-1]
                os3 = o_ap.rearrange("p (n two) -> p n two", two=2)
                n = sz // 2
                np_pool = pool_cols // 2
                nd = n - np_pool
                if nd:
                    nc.vector.tensor_add(
                        os3[:, :nd, :], os3[:, :nd, :], usw[:, :nd, :]
                    )
                if np_pool:
                    nc.gpsimd.tensor_add(
                        os3[:, nd:, :], os3[:, nd:, :], usw[:, nd:, :]
                    )

            row0 = 0
            for ci, nsl in enumerate(CHUNK_PLAN):
                first = ci == 0
                last = ci == len(CHUNK_PLAN) - 1
                cfree = nsl * FREE
                rows = x_d[row0 * 128 : (row0 + nsl) * 128, :]
                src = rows.rearrange("(j p) f -> p j f", j=nsl)
                xt = xin.tile([128, cfree], f16, tag="xt")
                if first:
                    # split the first load so compute can start after 0.25 MiB
                    assert nsl == 1
                    h = cfree // 2
                    nc.sync.dma_start(xt[:, :h], rows[:, :h])
                    nc.sync.dma_start(xt[:, h:], rows[:, h:])
                else:
                    nc.sync.dma_start(
                        xt[:].rearrange("p (j f) -> p j f", j=nsl), src
                    )

                w = wpool.tile([128, 2 * cfree], f16, tag="w")
                orows = o_d[row0 * 128 : (row0 + nsl) * 128, :]

                if first or last:
                    # head/tail chunk: process in free-dim halves (head: start
                    # computing after the first half-load; tail: overlap the
                    # final store with the second half's compute)
                    assert nsl == 1
                    part = cfree // 2
                    for hi in range(2):
                        lo = hi * part
                        merged_mul(w, xt, 1, fs=(lo, part))
                        swap_add(w, cfree, lo, part, 0)
                        nc.scalar.dma_start(
                            orows[:, lo : lo + part],
                            w[:, cfree + lo : cfree + lo + part],
                        )
                else:
                    merged_mul(w, xt, nsl)
                    pool_cols = int(cfree * POOL_ADD_FRAC) & ~1
                    swap_add(w, cfree, 0, cfree, pool_cols)
                    dst = orows.rearrange("(j p) f -> p j f", j=nsl)
                    nc.scalar.dma_start(
                        dst,
                        w[:, cfree:].rearrange("p (j f) -> p j f", j=nsl),
                    )
                row0 += nsl

    nc.compile()
    _prog_cache["nc"] = nc
    return nc


def _default_rotate_m(theta=10000.0):
    """Rebuild the reference's rotation buffer if the harness doesn't pass it."""
    half = D // 2
    try:  # replicate the reference's jax-f32 arithmetic exactly if possible
        import jax.numpy as jnp

        pos = np.asarray(jnp.arange(S, dtype=jnp.float32))
        inv_freq = np.asarray(
            theta ** (-(2.0 * jnp.arange(half, dtype=jnp.float32)) / D)
        )
        ang = np.asarray(pos[:, None] * inv_freq[None, :], dtype=np.float32)
        c, s = np.asarray(jnp.cos(ang)), np.asarray(jnp.sin(ang))
    except Exception:
        pos = np.arange(S, dtype=np.float32)
        exp = (-(2.0 * np.arange(half, dtype=np.float32)) / D).astype(np.float32)
        inv_freq = np.power(np.float32(theta), exp, dtype=np.float32)
        ang = (pos[:, None] * inv_freq[None, :]).astype(np.float32)
        c, s = np.cos(ang, dtype=np.float32), np.sin(ang, dtype=np.float32)
    idx = 2 * np.arange(half)
    r = np.zeros((S, D, D), dtype=np.float32)
    r[:, idx, idx] = c
    r[:, idx, idx + 1] = -s
    r[:, idx + 1, idx] = s
    r[:, idx + 1, idx + 1] = c
    return r


def _tables(token_positions, rotate_m):
    """Host-precompute the [128, FREE] fp16 A and B' tables.

    A[s,2k] = r[2k,2k], A[s,2k+1] = r[2k+1,2k+1]  (cos terms)
    B'[s,2k] = r[2k+1,2k], B'[s,2k+1] = r[2k,2k+1] (pre-pairswapped sin terms
    so that pairswap(x*B') lands the right products on the right lanes)."""
    if rotate_m is None:
        rotate_m = _default_rotate_m()
    r = np.asarray(rotate_m, dtype=np.float32)[np.asarray(token_positions)]
    idx = np.arange(D // 2) * 2
    a = r[:, idx, idx]            # x_even -> out_even
    b = r[:, idx, idx + 1]        # x_odd  -> out_even
    c = r[:, idx + 1, idx + 1]    # x_odd  -> out_odd
    d = r[:, idx + 1, idx]        # x_even -> out_odd
    A = np.empty((S, D), np.float32)
    A[:, 0::2] = a
    A[:, 1::2] = c
    Bp = np.empty((S, D), np.float32)
    Bp[:, 0::2] = d
    Bp[:, 1::2] = b
    return (
        np.ascontiguousarray(A.reshape(128, FREE)).astype(np.float16),
        np.ascontiguousarray(Bp.reshape(128, FREE)).astype(np.float16),
    )


def _in_maps(x, token_positions, rotate_m):
    ta, tb = _tables(token_positions, rotate_m)
    xs = np.asarray(x, dtype=np.float32).astype(np.float16).reshape(
        N_CORES, ROWS, FREE
    )
    xs = np.ascontiguousarray(xs)
    return [{"x": xs[i], "ta": ta, "tb": tb} for i in range(N_CORES)]


def _run(x, token_positions, rotate_m=None, trace=False, trace_cores=None):
    from concourse.bass_utils import run_bass_kernel_spmd

    nc = _build_program()
    in_maps = _in_maps(x, token_positions, rotate_m)
    res = run_bass_kernel_spmd(
        nc,
        in_maps,
        list(range(N_CORES)),
        trace=trace,
        trace_cores=trace_cores,
    )
    out = np.concatenate(
        [res.results[i]["out"].reshape(1, ROWS * FREE) for i in range(N_CORES)]
    ).reshape(B, H, S, D).astype(np.float32)
    return out, res


def kernel(x, token_positions, rotate_m=None, **_unused):
    out, _ = _run(x, token_positions, rotate_m, trace=False)
    return out
